# revision 5
# baseline (speedup 1.0000x reference)
"""Trainium2 Bass kernel for a 3x3 stride-1 pad-1 conv:
x (32,128,64,64) f32, weight (256,128,3,3) f32, bias (256,) f32
-> out (32,256,64,64) f32.

Strategy: data-parallel over batch across 8 NeuronCores (4 samples each).
Per core, the conv is 9 shifted matmuls accumulating in PSUM:
  out[co, hw] = sum_{kh,kw} W[co, :, kh, kw] @ x[:, h+kh-1, w+kw-1]
C_in=128 sits on the SBUF partition dim. Operands are cast to fp16
(full-rate PE mode, 2-byte stationary loads; conv rel-err ~4e-4).

Horizontal padding: three pre-shifted fp16 copies of each sample
(c0 = x shifted right w/ zero col 0, c1 = x, c2 = x shifted left w/
zero col 63) make every matmul RHS a fully contiguous [128, 512]
window. Vertical padding needs no stored zeros: edge tiles restrict
the matmul to a sub-range of the PSUM bank (taps ordered so the
first matmul of each accumulation group covers the full range, so
has_written semantics overwrite/accumulate correctly).

Outputs are stored as fp16 (halves store traffic; upcast on host).
"""

import numpy as np

import concourse.bass as bass
from concourse import bacc
import concourse.mybir as mybir
import concourse.tile as tile
from concourse.bass_utils import run_bass_kernel_spmd
from concourse.masks import make_identity

N_CORES = 8
B_FULL = 32
B_LOCAL = B_FULL // N_CORES  # 4
CI = 128
CO = 256
H = W = 64
ROWS = 8  # output rows per PSUM tile -> free dim 8*64 = 512
N_T = H // ROWS
F32 = mybir.dt.float32
F16 = mybir.dt.float16

# x0 arrives in three chunks so compute can start early. Tile t needs
# input rows 8t-1 .. 8t+8, so rows 0-9 unlock tile 0, rows 10-33 unlock
# tiles 1-3, rows 34-63 the rest.
S0_CHUNKS = [(0, 10), (10, 24), (34, 30)]


def build_nc():
    nc = bacc.Bacc()
    x_d = nc.dram_tensor("x", [B_LOCAL, CI, H, W], F32, kind="ExternalInput")
    w_d = nc.dram_tensor("weight", [CO, CI, 3, 3], F32, kind="ExternalInput")
    b_d = nc.dram_tensor("bias", [CO], F32, kind="ExternalInput")
    o_d = nc.dram_tensor("out", [B_LOCAL, CO, H, W], F16, kind="ExternalOutput")

    with tile.TileContext(nc) as tc:
        with (
            tc.tile_pool(name="const", bufs=1) as const,
            tc.tile_pool(name="xstage", bufs=B_LOCAL) as xstage,
            tc.tile_pool(name="c0", bufs=2) as c0pool,
            tc.tile_pool(name="c1", bufs=2) as c1pool,
            tc.tile_pool(name="c2", bufs=2) as c2pool,
            tc.tile_pool(name="obuf", bufs=12) as opool,
            tc.tile_pool(name="psum", bufs=6, space="PSUM") as pspool,
            tc.tile_pool(name="psum_tr", bufs=2, space="PSUM") as trpool,
        ):
            ident = const.tile([128, 128], F32)
            make_identity(nc, ident)
            # PE_HAM flips the clock gate 1.2->2.4 GHz after ~3.4us of
            # sustained PE activity; keep the PE busy on dummy transposes
            # while the weight DMA lands so the ramp starts immediately.
            for _ in range(6):
                warm = trpool.tile([128, 128], F32, tag="tr")
                nc.tensor.transpose(warm, ident, ident)

            # Weights ride the ACT HWDGE ring; x rides the SP ring, so the
            # two first-tile dependencies stream in parallel from t=0.
            w_raw = const.tile([128, 2, CI * 9], F32)
            w_v = w_d.rearrange("(cb cp) ci kh kw -> cp cb (ci kh kw)", cb=2)
            nc.scalar.dma_start(w_raw[:, 0], w_v[:, 0])

            x_v = x_d.rearrange("b c h w -> b c (h w)")
            stage0 = xstage.tile([128, H * W], F32)
            for r0, nr in S0_CHUNKS:
                nc.sync.dma_start(
                    stage0[:, r0 * W : (r0 + nr) * W],
                    x_v[0, :, r0 * W : (r0 + nr) * W],
                )

            nc.scalar.dma_start(w_raw[:, 1], w_v[:, 1])
            bias_sb = const.tile([128, 2], F32)
            nc.scalar.dma_start(bias_sb, b_d.rearrange("(cb cp) -> cp cb", cb=2))

            for b in range(1, B_LOCAL):
                st = xstage.tile([128, H * W], F32)
                nc.sync.dma_start(st, x_v[b])
                if b == 1:
                    stage1 = st
                elif b == 2:
                    stage2 = st
                else:
                    stage3 = st
            stages = [stage0, stage1, stage2, stage3]

            # Transpose each 128x128 (cb,kh,kw) weight slice on the PE and
            # cast to fp16: w_t[ci, cb*9+k, co_p].
            w_t = const.tile([128, 18, 128], F16)

            def transpose_cb(cb):
                w_cb = w_raw[:, cb, :].rearrange("p (ci k) -> p k ci", k=9)
                for k in range(9):
                    ptr = trpool.tile([128, 128], F32, tag="tr")
                    nc.tensor.transpose(ptr, w_cb[:, k, :], ident)
                    nc.vector.tensor_copy(w_t[:, cb * 9 + k, :], ptr)

            # Shifted fp16 copies of rows [r0, r0+nr) of sample b.
            # c0 = right-shift (zero col 0), c1 = identity, c2 = left-shift
            # (zero col 63); all flat contiguous copies + tiny edge fixes.
            def shift_copies(cs, st, r0, nr):
                a, b_ = r0 * W, (r0 + nr) * W
                c0, c1, c2 = cs
                c0f = c0.rearrange("p h w -> p (h w)")
                c1f = c1.rearrange("p h w -> p (h w)")
                c2f = c2.rearrange("p h w -> p (h w)")
                nc.vector.tensor_copy(c1f[:, a:b_], st[:, a:b_])
                nc.vector.tensor_copy(c0f[:, a + 1 : b_], st[:, a : b_ - 1])
                nc.vector.tensor_copy(c2f[:, a : b_ - 1], st[:, a + 1 : b_])
                nc.vector.memset(c0[:, r0 : r0 + nr, 0], 0.0)
                nc.vector.memset(c2[:, r0 : r0 + nr, W - 1], 0.0)

            def alloc_cs():
                return (
                    c0pool.tile([128, H, W], F16, name="c0", tag="c0"),
                    c1pool.tile([128, H, W], F16, name="c1", tag="c1"),
                    c2pool.tile([128, H, W], F16, name="c2", tag="c2"),
                )

            cs0 = alloc_cs()
            shift_copies(cs0, stage0, *S0_CHUNKS[0])
            transpose_cb(0)
            shift_copies(cs0, stage0, *S0_CHUNKS[1])
            transpose_cb(1)
            shift_copies(cs0, stage0, *S0_CHUNKS[2])

            all_cs = [cs0]
            for b in range(1, B_LOCAL):
                cs = alloc_cs()
                shift_copies(cs, stages[b], 0, H)
                all_cs.append(cs)

            o_v = o_d.rearrange("b (cb cp) h w -> b cb cp (h w)", cb=2)

            # Sample 0's tiles ordered by chunk arrival; cb0 first so the
            # cb1 weight transposes have time to land.
            sample0_order = (
                [(0, 0), (0, 1), (0, 2), (0, 3), (1, 0), (1, 1), (1, 2), (1, 3)]
                + [(0, t) for t in range(4, N_T)]
                + [(1, t) for t in range(4, N_T)]
            )
            std_order = [(cb, t) for cb in range(2) for t in range(N_T)]

            def conv_tile(b, cb, t):
                cs = all_cs[b]
                h0 = t * ROWS
                ps = pspool.tile([128, ROWS * W], F32)
                # Taps ordered so the first matmul covers the full PSUM
                # range (start=True clears the whole bank's has_written).
                kh_order = (1, 2, 0) if t == 0 else ((1, 0, 2) if t == N_T - 1 else (0, 1, 2))
                i = 0
                for kh in kh_order:
                    r0 = h0 + kh - 1
                    for kw in range(3):
                        w_ap = w_t[:, cb * 9 + kh * 3 + kw, :]
                        if r0 < 0:
                            nc.tensor.matmul(
                                ps[:, W:], w_ap, cs[kw][:, 0 : ROWS - 1, :],
                                start=(i == 0), stop=(i == 8),
                            )
                        elif r0 + ROWS > H:
                            nc.tensor.matmul(
                                ps[:, : (ROWS - 1) * W], w_ap,
                                cs[kw][:, r0:H, :],
                                start=(i == 0), stop=(i == 8),
                            )
                        else:
                            nc.tensor.matmul(
                                ps, w_ap, cs[kw][:, r0 : r0 + ROWS, :],
                                start=(i == 0), stop=(i == 8),
                            )
                        i += 1
                ob = opool.tile([128, ROWS * W], F16)
                nc.scalar.add(ob, ps, bias_sb[:, cb : cb + 1])
                nc.sync.dma_start(o_v[b, cb, :, h0 * W : (h0 + ROWS) * W], ob)

            for b in range(B_LOCAL):
                for cb, t in sample0_order if b == 0 else std_order:
                    conv_tile(b, cb, t)

    nc.finalize()
    return nc


def run(x: np.ndarray, weight: np.ndarray, bias: np.ndarray, **spmd_kwargs):
    x = np.ascontiguousarray(x, dtype=np.float32)
    weight = np.ascontiguousarray(weight, dtype=np.float32)
    bias = np.ascontiguousarray(bias, dtype=np.float32)

    nc = build_nc()
    in_maps = [
        {
            "x": x[c * B_LOCAL : (c + 1) * B_LOCAL],
            "weight": weight,
            "bias": bias,
        }
        for c in range(N_CORES)
    ]
    res = run_bass_kernel_spmd(
        nc, in_maps, core_ids=list(range(N_CORES)), **spmd_kwargs
    )
    out = np.concatenate(
        [np.asarray(r["out"]).astype(np.float32) for r in res.results], axis=0
    )
    return out, res


def kernel(x: np.ndarray, weight: np.ndarray, bias: np.ndarray) -> np.ndarray:
    out, _ = run(x, weight, bias)
    return out
